# revision 37
# baseline (speedup 1.0000x reference)
"""Trainium2 Bass kernel for a single-layer MHA + FFN transformer block.

Reference computation (x: [1, 4096, 768], 12 heads, dff=3072):
    qkv = (x @ w_qkv + b_qkv)  -> q, k, v
    scores = q k^T / sqrt(768) ; wei = softmax(scores)
    attn = wei @ v  (concat heads)
    h = gelu(attn @ w_ff1 + b_ff1)
    out = h @ w_ff2 + b_ff2

Sharding: sequence-parallel over the 4096 tokens across 8 NeuronCores
(512 rows each). Every core computes q/k/v for its own rows; k/v blocks
are exchanged with four pipelined fp8 AllGathers (k0 alone first so
attention starts earliest) and a tiny warm-up collective that absorbs
the rank barrier and collective-firmware spin-up.

Precision: weights and x are cast to bf16 on the host; q/k/v are fp8e4
on the wire and in SBUF (sqrt(1/sqrt(d)) of the score scale folded into
each of the q and k projection columns so fp8 stays in its normal
range); matmuls accumulate in fp32 PSUM. Softmax exp is split between
the scalar engine (table exp) and the vector engine (fused custom-DVE
cubic, logits are bounded by ~0.8); the denominator rides as a
ones-column in the packed v tiles; normalization uses the two-op
approximate reciprocal plus a K=1 broadcast matmul per head.

FFN2 computes the transposed output (out^T) so its 144 N=512 matmuls
share PSUM with FFN1 and fully interleave with it; the host transposes
each core's [768, 512] result back.
"""

import json as _json
import math

import numpy as np
import ml_dtypes

import concourse.bass as bass
import concourse.mybir as mybir
import concourse.tile as tile
from concourse.bass_utils import run_bass_kernel_spmd

# ---------------------------------------------------------------------------
# Workaround: the pinned walrus build only supports ONE embedded semaphore
# wait per instruction, but Tile's sem assigner attaches several. Split the
# excess onto standalone EventSemaphore instructions (pure waits) inserted
# just before the over-subscribed instruction (same engine => same program
# order, identical semantics).
# ---------------------------------------------------------------------------
_MAX_WAITS = 1
_ctr = [0]
if not getattr(bass.Bass, "_multiwait_patched", False):
    _orig_to_json_bytes = bass.Bass.to_json_bytes

    def _split_multiwait_json_bytes(self):
        bir = _json.loads(_orig_to_json_bytes(self))
        for f in bir["functions"]:
            for b in f["blocks"]:
                new_insts = []
                for inst in b["instructions"]:
                    si = inst.get("sync_info")
                    waits = si.get("on_wait", []) if si else []
                    if len(waits) > _MAX_WAITS:
                        excess, keep = waits[:-_MAX_WAITS], waits[-_MAX_WAITS:]
                        for k in range(0, len(excess), _MAX_WAITS):
                            _ctr[0] += 1
                            new_insts.append({
                                "debug": inst.get("debug", 0),
                                "engine": inst["engine"],
                                "ins": [], "outs": [],
                                "name": "I-waitsplit-%d" % _ctr[0],
                                "opcode": "EventSemaphore",
                                "sync_info": {"on_update": [],
                                              "on_wait": excess[k:k + _MAX_WAITS]},
                            })
                        si["on_wait"] = keep
                    new_insts.append(inst)
                b["instructions"] = new_insts
        return _json.dumps(bir).encode()

    bass.Bass.to_json_bytes = _split_multiwait_json_bytes
    bass.Bass._multiwait_patched = True

F32 = mybir.dt.float32
F32R = mybir.dt.float32r
BF16 = mybir.dt.bfloat16
FP8 = mybir.dt.float8e4
AFT = mybir.ActivationFunctionType

R = 8          # cores
T = 4096       # sequence length
TL = T // R    # rows per core (512)
D = 768
H = 12
HD = D // H    # 64
DFF = 4 * D    # 3072
P = 128
NDT = D // P   # 6 d-tiles
NTT = TL // P  # 4 local t-tiles
NFT = DFF // P  # 24 dff tiles
NPAIR = H // 2  # 6 head pairs
SCALE = 1.0 / math.sqrt(D)

PW = 2 * (HD + 1)          # 130: padded v width per pair ([v_h|1] x 2)
K_ELEMS = P * TL           # 65536 per-pair k^T payload elems (fp8, 1B)
V_ELEMS = TL * PW          # 66560 per-pair padded-v payload elems (bf16, 2B)
K_BYTES = K_ELEMS
V_BYTES = 2 * V_ELEMS
# pipelined AllGather batches: k0 alone so pair-0 scores start earliest
AG_GROUPS = [
    ("k", [0]),
    ("v", [0]),
    ("k", [1, 2]),
    ("v", [1, 2]),
    ("k", [3, 4, 5]),
    ("v", [3, 4, 5]),
]

# exp on [-0.8, 0.8] as ((s+K1)*s + K2)*(s*K3) + 1, rel err <= 0.46%
EXP_K1 = 3.3446521216989074
EXP_K2 = 6.493501417829298
EXP_K3 = 0.15487538281525948
# chunks (index mod 7) routed to the vector-engine exp; rest use scalar
EXP_DVE_RESIDUES = (1, 3, 5)

_NC_CACHE = {}


# ---------------------------------------------------------------------------
# Custom fused DVE op: cubic exp approximation in ONE vector instruction.
# out = ((s + K1)*s + K2) * (s*K3) + 1  ==  1 + c1 s + c2 s^2 + c3 s^3
# Registered into concourse.dve_ops at import time (repo is read-only).
# ---------------------------------------------------------------------------
def _register_exp3():
    from concourse import dve_ops as dops
    from concourse.dve_spec import Spec, Src0, C0, C1, C2, One, lower
    from concourse.dve_uop import DveOpSpec

    name = "EXP3_ANT"
    for op in dops.OPS:
        if op.name == name:
            return op

    def _ref(in0, in1, s0, s1, imm2):
        return (((in0 + s0) * in0 + s1) * (in0 * imm2) + 1.0).astype(np.float32)

    spec = Spec(body=((Src0 + C0) * Src0 + C1) * (Src0 * C2) + One,
                reference=_ref)
    row = dops._CUSTOM_DVE_ROW_BASE + len(dops.OPS)
    assert row < 0x20
    dops._SUB_OPCODE_FOR_NAME[name] = row
    shas = {}
    for ver in ("v3", "v4"):
        try:
            s = DveOpSpec(name=name, opcode=row, uops=lower(spec, ver=ver),
                          rd1_en=False)
            shas[ver] = s.sha(ver)
        except Exception:
            pass
    assert shas, "EXP3_ANT failed to lower for every DveVer"
    op = dops.DveOp(name, spec, subdim=False, uops_sha=shas)
    dops.OPS.append(op)
    return op


EXP3 = _register_exp3()


def _block_offsets(grp):
    """[(pair, offset_elems)] plus total elems for one AG group."""
    kind, pairs = grp
    sz = K_ELEMS if kind == "k" else V_ELEMS
    return [(p_, i * sz) for i, p_ in enumerate(pairs)], sz * len(pairs)


def _build_nc():
    nc = bass.Bass(num_devices=R)
    x = nc.declare_dram_parameter("x", [TL, D], BF16, isOutput=False)
    w_qkv = nc.declare_dram_parameter("w_qkv", [D, 3 * D], BF16, isOutput=False)
    b_qkv = nc.declare_dram_parameter("b_qkv", [3 * D], F32, isOutput=False)
    w_ff1 = nc.declare_dram_parameter("w_ff1", [D, DFF], BF16, isOutput=False)
    b_ff1 = nc.declare_dram_parameter("b_ff1", [DFF], F32, isOutput=False)
    w_ff2 = nc.declare_dram_parameter("w_ff2", [DFF, D], BF16, isOutput=False)
    b_ff2 = nc.declare_dram_parameter("b_ff2", [D], F32, isOutput=False)
    # transposed output; the host flips it back
    y = nc.declare_dram_parameter("y", [D, TL], F32, isOutput=True)

    from contextlib import ExitStack

    with tile.TileContext(nc) as tc, ExitStack() as top:
        const = top.enter_context(tc.tile_pool(name="const", bufs=1))
        dramp = top.enter_context(tc.tile_pool(name="dramp", bufs=1, space="DRAM"))
        persist = top.enter_context(tc.tile_pool(name="persist", bufs=1))

        ones_dram = nc.inline_tensor(np.ones((1, P), np.float32), name="ones_const")
        ones_row = const.tile([1, P], F32R, name="ones_row")
        nc.sync.dma_start(ones_row[:], ones_dram.ap().bitcast(F32R))

        bq_sb = const.tile([P, 3 * D // P], F32, name="bq_sb")
        nc.sync.dma_start(bq_sb[:], b_qkv.ap().rearrange("(o p) -> p o", p=P))
        bv_sb = const.tile([1, D], F32R, name="bv_sb")
        nc.sync.dma_start(bv_sb[:], b_qkv.ap()[None, 2 * D:3 * D].bitcast(F32R))
        b1_sb = const.tile([P, NFT], F32, name="b1_sb")
        nc.sync.dma_start(b1_sb[:], b_ff1.ap().rearrange("(o p) -> p o", p=P))
        b2_sb = const.tile([P, NDT], F32, name="b2_sb")
        nc.sync.dma_start(b2_sb[:], b_ff2.ap().rearrange("(o p) -> p o", p=P))

        # Tiny collective issued immediately: absorbs the rank barrier and
        # ncfw warm-up concurrently with the projection phase.
        warm_in = dramp.tile([64], BF16, name="warm_in")
        warm_out = dramp.tile([R * 64], BF16, addr_space="Shared",
                              name="warm_out")
        nc.vector.memset(warm_sb := const.tile([1, 64], BF16, name="warm_sb"), 0.0)
        nc.scalar.dma_start(warm_in[:].rearrange("(a b) -> a b", a=1), warm_sb[:])
        nc.gpsimd.collective_compute(
            "AllGather", mybir.AluOpType.bypass,
            replica_groups=[list(range(R))],
            ins=[warm_in[:]], outs=[warm_out[:]],
        )

        ag_ins, ag_outs, ag_offsets, ag_sizes = [], [], [], []
        for gi, grp in enumerate(AG_GROUPS):
            offs, total = _block_offsets(grp)
            dt_ = FP8 if grp[0] == "k" else BF16
            ag_offsets.append(offs)
            ag_sizes.append(total)
            ag_ins.append(dramp.tile([total], dt_, name=f"ag_in{gi}"))
            ag_outs.append(dramp.tile([R * total], dt_, addr_space="Shared",
                                      name=f"ag_out{gi}"))

        attnT = [persist.tile([P, TL], BF16, name=f"attnT{i}") for i in range(NDT)]
        hTp = top.enter_context(tc.tile_pool(name="hTp", bufs=1))
        hT = [hTp.tile([P, TL], BF16, name=f"hT{f}") for f in range(NFT)]
        w1p = top.enter_context(tc.tile_pool(name="w1p", bufs=3))
        w2p = top.enter_context(tc.tile_pool(name="w2p", bufs=3))

        kv_scope = top.enter_context(ExitStack())
        kvp = kv_scope.enter_context(tc.tile_pool(name="kvp", bufs=1))
        qT = [kvp.tile([P, TL], BF16, name=f"qT{p}") for p in range(NPAIR)]
        kT_loc = [kvp.tile([P, TL], FP8, name=f"kTl{p}") for p in range(NPAIR)]
        v_half = [[kvp.tile([P, 3 * PW], BF16, name=f"vp{t}_{h}")
                   for h in range(2)] for t in range(NTT)]
        kTf = [[kvp.tile([P, TL], FP8, name=f"kTf{p}_{r}") for r in range(R)]
               for p in range(NPAIR)]
        vf = [[kvp.tile([P, NTT * PW], BF16, name=f"vf{p}_{r}") for r in range(R)]
              for p in range(NPAIR)]

        # ------------------------------------------------------------------
        # Phase 1: x^T via DMA-xbar, QKV projections, pipelined AllGathers
        # ------------------------------------------------------------------
        with ExitStack() as ph1:
            xp = ph1.enter_context(tc.tile_pool(name="xp", bufs=1))
            wqp = ph1.enter_context(tc.tile_pool(name="wqp", bufs=3))
            psQ = ph1.enter_context(tc.tile_pool(name="psQ", bufs=2, space="PSUM"))

            xT = [xp.tile([P, TL], BF16, name=f"xT{d}") for d in range(NDT)]
            for d_ in range(NDT):
                eng = nc.sync if d_ % 2 == 0 else nc.scalar
                eng.dma_start_transpose(xT[d_][:],
                                        x.ap()[:, P * d_:P * (d_ + 1)])

            w_qkv_v = w_qkv.ap().rearrange("(o p) j -> p o j", p=P)

            def proj_jt(jt, out_tile, add_engine, dma_engine):
                """qkv^T tile for channel block jt: out[j, t] = W[:,j]^T x^T + b."""
                wq = wqp.tile([P, NDT, P], BF16, tag="wq", name="wq")
                dma_engine.dma_start(wq[:], w_qkv_v[:, :, P * jt:P * (jt + 1)])
                ps = psQ.tile([P, TL], F32, tag="psq", name="psq")
                for d_ in range(NDT):
                    nc.tensor.matmul(ps[:], wq[:, d_, :], xT[d_][:],
                                     start=(d_ == 0), stop=(d_ == NDT - 1))
                if add_engine == "vector":
                    nc.vector.tensor_scalar_add(out_tile[:], ps[:],
                                                bq_sb[:, jt:jt + 1])
                else:
                    nc.scalar.activation(out_tile[:], ps[:], AFT.Identity,
                                         bias=bq_sb[:, jt:jt + 1])

            def proj_v_half(o2):
                """v rows for heads [6*o2, 6*o2+6) into padded v_half tiles."""
                sl = slice(384 * o2, 384 * (o2 + 1))
                for tt in range(NTT):
                    ps = psQ.tile([P, TL], F32, tag="psq", name="psq")
                    for d_ in range(NDT):
                        nc.tensor.matmul(ps[:, :384],
                                         xT[d_][:, P * tt:P * (tt + 1)],
                                         wv[:, d_, sl],
                                         start=(d_ == 0), stop=False)
                    nc.tensor.matmul(ps[:, :384], ones_row[:], bv_sb[:, sl],
                                     start=False, stop=True)
                    vdst = v_half[tt][o2].rearrange("p (h e) -> p h e", e=HD + 1)
                    nc.vector.tensor_copy(
                        vdst[:, :, 0:HD],
                        ps[:, :384].rearrange("p (h e) -> p h e", e=HD))
                    nc.vector.memset(vdst[:, :, HD:HD + 1], 1.0)

            def stage(gi, kind, p_, off):
                if kind == "k":
                    ag_k = ag_ins[gi][off:off + K_ELEMS].rearrange(
                        "(a b) -> a b", b=TL)
                    nc.scalar.dma_start(ag_k[:, :], kT_loc[p_][:])
                else:
                    ag_v = ag_ins[gi][off:off + V_ELEMS].rearrange(
                        "(t c) -> t c", c=PW)
                    half, pp = divmod(p_, 3)
                    for tt in range(NTT):
                        nc.scalar.dma_start(
                            ag_v[P * tt:P * (tt + 1), :],
                            v_half[tt][half][:, PW * pp:PW * (pp + 1)])

            def kick(gi):
                nc.gpsimd.collective_compute(
                    "AllGather", mybir.AluOpType.bypass,
                    replica_groups=[list(range(R))],
                    ins=[ag_ins[gi][:]], outs=[ag_outs[gi][:]],
                )

            wv = xp.tile([P, NDT, D], BF16, name="wv")
            nc.scalar.dma_start(wv[:], w_qkv_v[:, :, 2 * D:3 * D])

            # g0: k0 | g1: v0 | g2: k1,k2 | g3: v1,v2 | g4: k3-5 | g5: v3-5
            proj_jt(NDT + 0, kT_loc[0], "scalar", nc.sync)
            stage(0, "k", 0, 0)
            kick(0)
            proj_v_half(0)
            stage(1, "v", 0, 0)
            kick(1)
            for i_, p_ in enumerate((1, 2)):
                proj_jt(NDT + p_, kT_loc[p_], "scalar", nc.sync)
                stage(2, "k", p_, ag_offsets[2][i_][1])
            kick(2)
            for i_, p_ in enumerate((1, 2)):
                stage(3, "v", p_, ag_offsets[3][i_][1])
            kick(3)
            proj_v_half(1)
            for i_, p_ in enumerate((3, 4, 5)):
                proj_jt(NDT + p_, kT_loc[p_], "scalar", nc.sync)
                stage(4, "k", p_, ag_offsets[4][i_][1])
            kick(4)
            for i_, p_ in enumerate((3, 4, 5)):
                stage(5, "v", p_, ag_offsets[5][i_][1])
            kick(5)

            # q projections overlap the collectives
            for p_ in range(NPAIR):
                proj_jt(p_, qT[p_], "vector", nc.sync)

            # keep-warm filler: bridges the PE-idle window while the first
            # AllGather completes so the HAM clock gate stays at 2.4 GHz.
            for wi in range(12):
                psw = psQ.tile([P, TL], F32, tag="psq", name="psw")
                nc.tensor.matmul(psw[:], xT[0][:, 0:P], xT[wi % NDT][:],
                                 start=True, stop=True)

            # AllGather returns: per (pair, rank) tiles; k on the sync ring,
            # v on the gpsimd ring.
            for gi, grp in enumerate(AG_GROUPS):
                kind = grp[0]
                n = ag_sizes[gi]
                ago = ag_outs[gi].rearrange("(r e) -> r e", e=n)
                for p_, off in ag_offsets[gi]:
                    for r in range(R):
                        if kind == "k":
                            src_k = ago[r, off:off + K_ELEMS].rearrange(
                                "(a b) -> a b", b=TL)
                            nc.sync.dma_start(kTf[p_][r][:], src_k)
                        else:
                            src_v = ago[r, off:off + V_ELEMS].rearrange(
                                "(s pi2 c) -> pi2 s c", pi2=P, c=PW)
                            dst_v = vf[p_][r].rearrange("p (s c) -> p s c", c=PW)
                            nc.gpsimd.dma_start(dst_v[:], src_v)

        # ------------------------------------------------------------------
        # Phase 2: attention, one head pair at a time
        # ------------------------------------------------------------------
        with ExitStack() as ph2:
            scp = ph2.enter_context(tc.tile_pool(name="scp", bufs=3, space="PSUM"))
            accp = ph2.enter_context(tc.tile_pool(name="accp", bufs=2, space="PSUM"))
            weip = ph2.enter_context(tc.tile_pool(name="weip", bufs=4))
            tailp = ph2.enter_context(tc.tile_pool(name="tailp", bufs=2))

            for p_ in range(NPAIR):
                acc0 = accp.tile([HD + 1, TL], F32, tag="acc", name="acc0")
                acc1 = accp.tile([HD + 1, TL], F32, tag="acc", name="acc1")
                for c in range(R * NTT):
                    r, s = divmod(c, NTT)
                    kt = kTf[p_][r]
                    sc = scp.tile([P, 2 * TL], F32, tag="sc", name="sc")
                    nc.tensor.matmul(sc[:, 0:TL],
                                     kt[0:HD, P * s:P * (s + 1)],
                                     qT[p_][0:HD, :], start=True, stop=True)
                    nc.tensor.matmul(sc[:, TL:2 * TL],
                                     kt[HD:P, P * s:P * (s + 1)],
                                     qT[p_][HD:P, :], start=True, stop=True)
                    wei = weip.tile([P, 2 * TL], BF16, tag="wei", name="wei")
                    if c % 7 in EXP_DVE_RESIDUES:
                        nc.vector._custom_dve(EXP3, out=wei[:], in0=sc[:],
                                              s0=EXP_K1, s1=EXP_K2, imm2=EXP_K3)
                    else:
                        nc.scalar.activation(wei[:], sc[:], AFT.Exp)
                    vt = vf[p_][r]
                    nc.tensor.matmul(acc0[:],
                                     vt[:, PW * s:PW * s + HD + 1],
                                     wei[:, 0:TL],
                                     start=(c == 0), stop=(c == R * NTT - 1))
                    nc.tensor.matmul(acc1[:],
                                     vt[:, PW * s + HD + 1:PW * (s + 1)],
                                     wei[:, TL:2 * TL],
                                     start=(c == 0), stop=(c == R * NTT - 1))

                # softmax tail: den -> 1/den -> broadcast -> scale
                from concourse.dve_ops import (
                    RECIPROCAL_APPROX_FAST, RECIPROCAL_APPROX_NR,
                    RECIP_APPROX_FAST_CONSTS as _RC)
                for hh, acc in ((0, acc0), (1, acc1)):
                    denc = tailp.tile([1, TL], F32, tag="denc", name="denc")
                    nc.vector.tensor_copy(denc[:], acc[HD:HD + 1, :])
                    den = tailp.tile([1, TL], F32R, tag="den", name="den")
                    scr = tailp.tile([1, TL], F32, tag="scr", name="scr")
                    nc.vector._custom_dve(
                        RECIPROCAL_APPROX_FAST, out=scr[:],
                        in0=denc[:], s0=_RC["s0"], s1=_RC["s1"],
                        imm2=_RC["imm2"])
                    nc.vector._custom_dve(
                        RECIPROCAL_APPROX_NR, out=den[:],
                        in0=denc[:], in1=scr[:], s0=2.0)
                    bc = scp.tile([HD, TL], F32, tag="sc", name="bc")
                    nc.tensor.matmul(bc[:], ones_row[:, 0:HD], den[:],
                                     start=True, stop=True)
                    recb = tailp.tile([HD, TL], F32, tag="recb", name="recb")
                    nc.vector.tensor_copy(recb[:], bc[:])
                    nc.vector.tensor_tensor(
                        attnT[p_][HD * hh:HD * (hh + 1), :], acc[0:HD, :],
                        recb[:], mybir.AluOpType.mult)

        kv_scope.close()

        # ------------------------------------------------------------------
        # Phase 3: FFN1 and FFN2 interleaved.
        #   h^T[f, t] = gelu(W1^T attn^T + b1)      (per f-tile)
        #   out^T[o, t] += W2[f, o]^T h^T[f, t]     (accumulated over f)
        # ------------------------------------------------------------------
        w_ff1_v = w_ff1.ap().rearrange("(o p) f -> p o f", p=P)
        w_ff2_v = w_ff2.ap().rearrange("(o p) d -> p o d", p=P)
        with ExitStack() as ph3:
            ps1 = ph3.enter_context(tc.tile_pool(name="ps1", bufs=2, space="PSUM"))
            ps2 = ph3.enter_context(tc.tile_pool(name="ps2", bufs=1, space="PSUM"))
            outp = ph3.enter_context(tc.tile_pool(name="outp", bufs=2))
            acc2 = [ps2.tile([P, TL], F32, name=f"acc2_{g}") for g in range(NDT)]
            for ft in range(NFT):
                w1 = w1p.tile([P, NDT, P], BF16, tag="w1", name="w1")
                nc.gpsimd.dma_start(w1[:], w_ff1_v[:, :, P * ft:P * (ft + 1)])
                ps = ps1.tile([P, TL], F32, tag="ps1t", name="ps1t")
                for d_ in range(NDT):
                    nc.tensor.matmul(ps[:], w1[:, d_, :], attnT[d_][:],
                                     start=(d_ == 0), stop=(d_ == NDT - 1))
                nc.scalar.activation(hT[ft][:], ps[:], AFT.Gelu,
                                     bias=b1_sb[:, ft:ft + 1])
                w2 = w2p.tile([P, D], BF16, tag="w2", name="w2")
                nc.gpsimd.dma_start(w2[:], w_ff2_v[:, ft, :])
                for ot in range(NDT):
                    nc.tensor.matmul(acc2[ot][:],
                                     w2[:, P * ot:P * (ot + 1)],
                                     hT[ft][:],
                                     start=(ft == 0), stop=(ft == NFT - 1))
            for ot in range(NDT):
                outT = outp.tile([P, TL], F32, tag="outT", name="outT")
                nc.vector.tensor_scalar_add(outT[:], acc2[ot][:],
                                            b2_sb[:, ot:ot + 1])
                nc.scalar.dma_start(y.ap()[P * ot:P * (ot + 1), :], outT[:])

    # The Tile path never runs bacc's codegen_inst_isa_subclasses pass, so
    # custom-DVE ISA wrappers would serialize with empty instruction bytes
    # ("ISA wrong length" in walrus). Lower them in place here.
    import concourse.bass_isa as bass_isa
    for func in nc.m.functions:
        for blk in func.blocks:
            i = 0
            while i < len(blk.instructions):
                inst = blk.instructions[i]
                if isinstance(inst, bass_isa.InstCustomDveAnt):
                    lowered = mybir.codegen_inst_isa_one(inst, nc._state, nc.isa)
                    assert isinstance(lowered, list)
                    del nc.inst_map[inst.name]
                    blk.instructions[i:i + 1] = lowered
                    for li in lowered:
                        nc.inst_map[li.name] = li
                    i += len(lowered)
                else:
                    i += 1

    return nc


def _get_nc():
    if "nc" not in _NC_CACHE:
        _NC_CACHE["nc"] = _build_nc()
    return _NC_CACHE["nc"]


def run_sharded(inputs, **run_kwargs):
    """Run the SPMD kernel; returns (full_output [1,4096,768], BassKernelResults)."""
    x = np.asarray(inputs["x"], dtype=np.float32)
    assert x.shape == (1, T, D), x.shape

    w_qkv = np.asarray(inputs["w_qkv"], dtype=np.float32).copy()
    b_qkv = np.asarray(inputs["b_qkv"], dtype=np.float32).copy()
    # fold the score scale into the q projection (bf16); k stays unscaled so
    # its fp8 values sit in the format's normal range
    w_qkv[:, 0:D] *= SCALE
    b_qkv[0:D] *= SCALE

    common = {
        "w_qkv": np.ascontiguousarray(w_qkv.astype(ml_dtypes.bfloat16)),
        "b_qkv": np.ascontiguousarray(b_qkv),
        "w_ff1": np.ascontiguousarray(
            np.asarray(inputs["w_ff1"], dtype=np.float32).astype(ml_dtypes.bfloat16)),
        "b_ff1": np.ascontiguousarray(np.asarray(inputs["b_ff1"], dtype=np.float32)),
        "w_ff2": np.ascontiguousarray(
            np.asarray(inputs["w_ff2"], dtype=np.float32).astype(ml_dtypes.bfloat16)),
        "b_ff2": np.ascontiguousarray(np.asarray(inputs["b_ff2"], dtype=np.float32)),
    }
    x_bf = x.astype(ml_dtypes.bfloat16)
    in_maps = []
    for r in range(R):
        m = dict(common)
        m["x"] = np.ascontiguousarray(x_bf[0, TL * r:TL * (r + 1), :])
        in_maps.append(m)
    nc = _get_nc()
    res = run_bass_kernel_spmd(nc, in_maps, core_ids=list(range(R)), **run_kwargs)
    out = np.concatenate([res.results[r]["y"].T for r in range(R)], axis=0)
    return out.reshape(1, T, D), res


def kernel(**inputs):
    out, _ = run_sharded(inputs)
    return out


# revision 38
# speedup vs baseline: 1.0277x; 1.0277x over previous
"""Trainium2 Bass kernel for a single-layer MHA + FFN transformer block.

Reference computation (x: [1, 4096, 768], 12 heads, dff=3072):
    qkv = (x @ w_qkv + b_qkv)  -> q, k, v
    scores = q k^T / sqrt(768) ; wei = softmax(scores)
    attn = wei @ v  (concat heads)
    h = gelu(attn @ w_ff1 + b_ff1)
    out = h @ w_ff2 + b_ff2

Sharding: sequence-parallel over the 4096 tokens across 8 NeuronCores
(512 rows each). Every core computes q/k/v for its own rows; k/v blocks
are exchanged with four pipelined fp8 AllGathers (k0 alone first so
attention starts earliest) and a tiny warm-up collective that absorbs
the rank barrier and collective-firmware spin-up.

Precision: weights and x are cast to bf16 on the host; q/k/v are fp8e4
on the wire and in SBUF (sqrt(1/sqrt(d)) of the score scale folded into
each of the q and k projection columns so fp8 stays in its normal
range); matmuls accumulate in fp32 PSUM. Softmax exp is split between
the scalar engine (table exp) and the vector engine (fused custom-DVE
cubic, logits are bounded by ~0.8); the denominator rides as a
ones-column in the packed v tiles; normalization uses the two-op
approximate reciprocal plus a K=1 broadcast matmul per head.

FFN2 computes the transposed output (out^T) so its 144 N=512 matmuls
share PSUM with FFN1 and fully interleave with it; the host transposes
each core's [768, 512] result back.
"""

import json as _json
import math

import numpy as np
import ml_dtypes

import concourse.bass as bass
import concourse.mybir as mybir
import concourse.tile as tile
from concourse.bass_utils import run_bass_kernel_spmd

# ---------------------------------------------------------------------------
# Workaround: the pinned walrus build only supports ONE embedded semaphore
# wait per instruction, but Tile's sem assigner attaches several. Split the
# excess onto standalone EventSemaphore instructions (pure waits) inserted
# just before the over-subscribed instruction (same engine => same program
# order, identical semantics).
# ---------------------------------------------------------------------------
_MAX_WAITS = 1
_ctr = [0]
if not getattr(bass.Bass, "_multiwait_patched", False):
    _orig_to_json_bytes = bass.Bass.to_json_bytes

    def _split_multiwait_json_bytes(self):
        bir = _json.loads(_orig_to_json_bytes(self))
        for f in bir["functions"]:
            for b in f["blocks"]:
                new_insts = []
                for inst in b["instructions"]:
                    si = inst.get("sync_info")
                    waits = si.get("on_wait", []) if si else []
                    if len(waits) > _MAX_WAITS:
                        excess, keep = waits[:-_MAX_WAITS], waits[-_MAX_WAITS:]
                        for k in range(0, len(excess), _MAX_WAITS):
                            _ctr[0] += 1
                            new_insts.append({
                                "debug": inst.get("debug", 0),
                                "engine": inst["engine"],
                                "ins": [], "outs": [],
                                "name": "I-waitsplit-%d" % _ctr[0],
                                "opcode": "EventSemaphore",
                                "sync_info": {"on_update": [],
                                              "on_wait": excess[k:k + _MAX_WAITS]},
                            })
                        si["on_wait"] = keep
                    new_insts.append(inst)
                b["instructions"] = new_insts
        return _json.dumps(bir).encode()

    bass.Bass.to_json_bytes = _split_multiwait_json_bytes
    bass.Bass._multiwait_patched = True

F32 = mybir.dt.float32
F32R = mybir.dt.float32r
BF16 = mybir.dt.bfloat16
FP8 = mybir.dt.float8e4
AFT = mybir.ActivationFunctionType

R = 8          # cores
T = 4096       # sequence length
TL = T // R    # rows per core (512)
D = 768
H = 12
HD = D // H    # 64
DFF = 4 * D    # 3072
P = 128
NDT = D // P   # 6 d-tiles
NTT = TL // P  # 4 local t-tiles
NFT = DFF // P  # 24 dff tiles
NPAIR = H // 2  # 6 head pairs
SCALE = 1.0 / math.sqrt(D)

PW = 2 * (HD + 1)          # 130: padded v width per pair ([v_h|1] x 2)
K_ELEMS = P * TL           # 65536 per-pair k^T payload elems (fp8, 1B)
V_ELEMS = TL * PW          # 66560 per-pair padded-v payload elems (bf16, 2B)
K_BYTES = K_ELEMS
V_BYTES = 2 * V_ELEMS
# pipelined AllGather batches: k0 alone so pair-0 scores start earliest
AG_GROUPS = [
    ("k", [0]),
    ("v", [0]),
    ("k", [1, 2]),
    ("v", [1, 2]),
    ("k", [3, 4, 5]),
    ("v", [3, 4, 5]),
]

# exp on [-0.8, 0.8] as ((s+K1)*s + K2)*(s*K3) + 1, rel err <= 0.46%
EXP_K1 = 3.3446521216989074
EXP_K2 = 6.493501417829298
EXP_K3 = 0.15487538281525948
# chunks (index mod 7) routed to the vector-engine exp; rest use scalar
EXP_DVE_RESIDUES = (1, 3, 5)

_NC_CACHE = {}


# ---------------------------------------------------------------------------
# Custom fused DVE op: cubic exp approximation in ONE vector instruction.
# out = ((s + K1)*s + K2) * (s*K3) + 1  ==  1 + c1 s + c2 s^2 + c3 s^3
# Registered into concourse.dve_ops at import time (repo is read-only).
# ---------------------------------------------------------------------------
def _register_exp3():
    from concourse import dve_ops as dops
    from concourse.dve_spec import Spec, Src0, C0, C1, C2, One, lower
    from concourse.dve_uop import DveOpSpec

    name = "EXP3_ANT"
    for op in dops.OPS:
        if op.name == name:
            return op

    def _ref(in0, in1, s0, s1, imm2):
        return (((in0 + s0) * in0 + s1) * (in0 * imm2) + 1.0).astype(np.float32)

    spec = Spec(body=((Src0 + C0) * Src0 + C1) * (Src0 * C2) + One,
                reference=_ref)
    row = dops._CUSTOM_DVE_ROW_BASE + len(dops.OPS)
    assert row < 0x20
    dops._SUB_OPCODE_FOR_NAME[name] = row
    shas = {}
    for ver in ("v3", "v4"):
        try:
            s = DveOpSpec(name=name, opcode=row, uops=lower(spec, ver=ver),
                          rd1_en=False)
            shas[ver] = s.sha(ver)
        except Exception:
            pass
    assert shas, "EXP3_ANT failed to lower for every DveVer"
    op = dops.DveOp(name, spec, subdim=False, uops_sha=shas)
    dops.OPS.append(op)
    return op


EXP3 = _register_exp3()


def _block_offsets(grp):
    """[(pair, offset_elems)] plus total elems for one AG group."""
    kind, pairs = grp
    sz = K_ELEMS if kind == "k" else V_ELEMS
    return [(p_, i * sz) for i, p_ in enumerate(pairs)], sz * len(pairs)


def _build_nc():
    nc = bass.Bass(num_devices=R)
    x = nc.declare_dram_parameter("x", [TL, D], BF16, isOutput=False)
    w_qkv = nc.declare_dram_parameter("w_qkv", [D, 3 * D], BF16, isOutput=False)
    b_qkv = nc.declare_dram_parameter("b_qkv", [3 * D], F32, isOutput=False)
    w_ff1 = nc.declare_dram_parameter("w_ff1", [D, DFF], BF16, isOutput=False)
    b_ff1 = nc.declare_dram_parameter("b_ff1", [DFF], F32, isOutput=False)
    w_ff2 = nc.declare_dram_parameter("w_ff2", [DFF, D], BF16, isOutput=False)
    b_ff2 = nc.declare_dram_parameter("b_ff2", [D], F32, isOutput=False)
    # transposed output; the host flips it back
    y = nc.declare_dram_parameter("y", [D, TL], F32, isOutput=True)

    from contextlib import ExitStack

    with tile.TileContext(nc) as tc, ExitStack() as top:
        const = top.enter_context(tc.tile_pool(name="const", bufs=1))
        dramp = top.enter_context(tc.tile_pool(name="dramp", bufs=1, space="DRAM"))
        persist = top.enter_context(tc.tile_pool(name="persist", bufs=1))

        ones_dram = nc.inline_tensor(np.ones((1, P), np.float32), name="ones_const")
        ones_row = const.tile([1, P], F32R, name="ones_row")
        nc.sync.dma_start(ones_row[:], ones_dram.ap().bitcast(F32R))

        bq_sb = const.tile([P, 3 * D // P], F32, name="bq_sb")
        nc.sync.dma_start(bq_sb[:], b_qkv.ap().rearrange("(o p) -> p o", p=P))
        bv_sb = const.tile([1, D], F32R, name="bv_sb")
        nc.sync.dma_start(bv_sb[:], b_qkv.ap()[None, 2 * D:3 * D].bitcast(F32R))
        b1_sb = const.tile([P, NFT], F32, name="b1_sb")
        nc.sync.dma_start(b1_sb[:], b_ff1.ap().rearrange("(o p) -> p o", p=P))
        b2_sb = const.tile([P, NDT], F32, name="b2_sb")
        nc.sync.dma_start(b2_sb[:], b_ff2.ap().rearrange("(o p) -> p o", p=P))

        # Tiny collective issued immediately: absorbs the rank barrier and
        # ncfw warm-up concurrently with the projection phase.
        warm_in = dramp.tile([64], BF16, name="warm_in")
        warm_out = dramp.tile([R * 64], BF16, addr_space="Shared",
                              name="warm_out")
        nc.vector.memset(warm_sb := const.tile([1, 64], BF16, name="warm_sb"), 0.0)
        nc.scalar.dma_start(warm_in[:].rearrange("(a b) -> a b", a=1), warm_sb[:])
        nc.gpsimd.collective_compute(
            "AllGather", mybir.AluOpType.bypass,
            replica_groups=[list(range(R))],
            ins=[warm_in[:]], outs=[warm_out[:]],
        )

        ag_ins, ag_outs, ag_offsets, ag_sizes = [], [], [], []
        for gi, grp in enumerate(AG_GROUPS):
            offs, total = _block_offsets(grp)
            dt_ = FP8 if grp[0] == "k" else BF16
            ag_offsets.append(offs)
            ag_sizes.append(total)
            ag_ins.append(dramp.tile([total], dt_, name=f"ag_in{gi}"))
            ag_outs.append(dramp.tile([R * total], dt_, addr_space="Shared",
                                      name=f"ag_out{gi}"))

        attnT = [persist.tile([P, TL], BF16, name=f"attnT{i}") for i in range(NDT)]
        hTp = top.enter_context(tc.tile_pool(name="hTp", bufs=1))
        hT = [hTp.tile([P, TL], BF16, name=f"hT{f}") for f in range(NFT)]
        w1p = top.enter_context(tc.tile_pool(name="w1p", bufs=3))
        w2p = top.enter_context(tc.tile_pool(name="w2p", bufs=3))

        kv_scope = top.enter_context(ExitStack())
        kvp = kv_scope.enter_context(tc.tile_pool(name="kvp", bufs=1))
        qT = [kvp.tile([P, TL], BF16, name=f"qT{p}") for p in range(NPAIR)]
        kT_loc = [kvp.tile([P, TL], FP8, name=f"kTl{p}") for p in range(NPAIR)]
        v_half = [[kvp.tile([P, 3 * PW], BF16, name=f"vp{t}_{h}")
                   for h in range(2)] for t in range(NTT)]
        kTf = [[kvp.tile([P, TL], FP8, name=f"kTf{p}_{r}") for r in range(R)]
               for p in range(NPAIR)]
        vf = [[kvp.tile([P, NTT * PW], BF16, name=f"vf{p}_{r}") for r in range(R)]
              for p in range(NPAIR)]

        # ------------------------------------------------------------------
        # Phase 1: x^T via DMA-xbar, QKV projections, pipelined AllGathers
        # ------------------------------------------------------------------
        with ExitStack() as ph1:
            xp = ph1.enter_context(tc.tile_pool(name="xp", bufs=1))
            wqp = ph1.enter_context(tc.tile_pool(name="wqp", bufs=3))
            psQ = ph1.enter_context(tc.tile_pool(name="psQ", bufs=2, space="PSUM"))

            xT = [xp.tile([P, TL], BF16, name=f"xT{d}") for d in range(NDT)]
            for d_ in range(NDT):
                eng = nc.sync if d_ % 2 == 0 else nc.scalar
                eng.dma_start_transpose(xT[d_][:],
                                        x.ap()[:, P * d_:P * (d_ + 1)])

            w_qkv_v = w_qkv.ap().rearrange("(o p) j -> p o j", p=P)

            def proj_jt(jt, out_tile, add_engine, dma_engine):
                """qkv^T tile for channel block jt: out[j, t] = W[:,j]^T x^T + b."""
                wq = wqp.tile([P, NDT, P], BF16, tag="wq", name="wq")
                dma_engine.dma_start(wq[:], w_qkv_v[:, :, P * jt:P * (jt + 1)])
                ps = psQ.tile([P, TL], F32, tag="psq", name="psq")
                for d_ in range(NDT):
                    nc.tensor.matmul(ps[:], wq[:, d_, :], xT[d_][:],
                                     start=(d_ == 0), stop=(d_ == NDT - 1))
                if add_engine == "vector":
                    nc.vector.tensor_scalar_add(out_tile[:], ps[:],
                                                bq_sb[:, jt:jt + 1])
                else:
                    nc.scalar.activation(out_tile[:], ps[:], AFT.Identity,
                                         bias=bq_sb[:, jt:jt + 1])

            def proj_v_half(o2):
                """v rows for heads [6*o2, 6*o2+6) into padded v_half tiles."""
                sl = slice(384 * o2, 384 * (o2 + 1))
                for tt in range(NTT):
                    ps = psQ.tile([P, TL], F32, tag="psq", name="psq")
                    for d_ in range(NDT):
                        nc.tensor.matmul(ps[:, :384],
                                         xT[d_][:, P * tt:P * (tt + 1)],
                                         wv[:, d_, sl],
                                         start=(d_ == 0), stop=False)
                    nc.tensor.matmul(ps[:, :384], ones_row[:], bv_sb[:, sl],
                                     start=False, stop=True)
                    vdst = v_half[tt][o2].rearrange("p (h e) -> p h e", e=HD + 1)
                    nc.vector.tensor_copy(
                        vdst[:, :, 0:HD],
                        ps[:, :384].rearrange("p (h e) -> p h e", e=HD))
                    nc.vector.memset(vdst[:, :, HD:HD + 1], 1.0)

            def stage(gi, kind, p_, off):
                if kind == "k":
                    ag_k = ag_ins[gi][off:off + K_ELEMS].rearrange(
                        "(a b) -> a b", b=TL)
                    nc.scalar.dma_start(ag_k[:, :], kT_loc[p_][:])
                else:
                    ag_v = ag_ins[gi][off:off + V_ELEMS].rearrange(
                        "(t c) -> t c", c=PW)
                    half, pp = divmod(p_, 3)
                    for tt in range(NTT):
                        nc.scalar.dma_start(
                            ag_v[P * tt:P * (tt + 1), :],
                            v_half[tt][half][:, PW * pp:PW * (pp + 1)])

            def kick(gi):
                nc.gpsimd.collective_compute(
                    "AllGather", mybir.AluOpType.bypass,
                    replica_groups=[list(range(R))],
                    ins=[ag_ins[gi][:]], outs=[ag_outs[gi][:]],
                )

            wv = xp.tile([P, NDT, D], BF16, name="wv")
            nc.scalar.dma_start(wv[:], w_qkv_v[:, :, 2 * D:3 * D])

            # g0: k0 | g1: v0 | g2: k1,k2 | g3: v1,v2 | g4: k3-5 | g5: v3-5
            proj_jt(NDT + 0, kT_loc[0], "scalar", nc.sync)
            stage(0, "k", 0, 0)
            kick(0)
            proj_v_half(0)
            stage(1, "v", 0, 0)
            kick(1)
            for i_, p_ in enumerate((1, 2)):
                proj_jt(NDT + p_, kT_loc[p_], "scalar", nc.sync)
                stage(2, "k", p_, ag_offsets[2][i_][1])
            kick(2)
            for i_, p_ in enumerate((1, 2)):
                stage(3, "v", p_, ag_offsets[3][i_][1])
            kick(3)
            proj_v_half(1)
            for i_, p_ in enumerate((3, 4, 5)):
                proj_jt(NDT + p_, kT_loc[p_], "scalar", nc.sync)
                stage(4, "k", p_, ag_offsets[4][i_][1])
            kick(4)
            for i_, p_ in enumerate((3, 4, 5)):
                stage(5, "v", p_, ag_offsets[5][i_][1])
            kick(5)

            # q projections overlap the collectives
            for p_ in range(NPAIR):
                proj_jt(p_, qT[p_], "vector", nc.sync)

            # keep-warm filler: bridges the PE-idle window while the first
            # AllGather completes so the HAM clock gate stays at 2.4 GHz.
            for wi in range(12):
                psw = psQ.tile([P, TL], F32, tag="psq", name="psw")
                nc.tensor.matmul(psw[:], xT[0][:, 0:P], xT[wi % NDT][:],
                                 start=True, stop=True)

            # AllGather returns: per (pair, rank) tiles; k on the sync ring,
            # v on the gpsimd ring.
            for gi, grp in enumerate(AG_GROUPS):
                kind = grp[0]
                n = ag_sizes[gi]
                ago = ag_outs[gi].rearrange("(r e) -> r e", e=n)
                for p_, off in ag_offsets[gi]:
                    for r in range(R):
                        if kind == "k":
                            src_k = ago[r, off:off + K_ELEMS].rearrange(
                                "(a b) -> a b", b=TL)
                            nc.sync.dma_start(kTf[p_][r][:], src_k)
                        else:
                            src_v = ago[r, off:off + V_ELEMS].rearrange(
                                "(s pi2 c) -> pi2 s c", pi2=P, c=PW)
                            dst_v = vf[p_][r].rearrange("p (s c) -> p s c", c=PW)
                            nc.gpsimd.dma_start(dst_v[:], src_v)

        # ------------------------------------------------------------------
        # Phase 2: attention, one head pair at a time
        # ------------------------------------------------------------------
        with ExitStack() as ph2:
            scp = ph2.enter_context(tc.tile_pool(name="scp", bufs=3, space="PSUM"))
            accp = ph2.enter_context(tc.tile_pool(name="accp", bufs=2, space="PSUM"))
            weip = ph2.enter_context(tc.tile_pool(name="weip", bufs=6))
            tailp = ph2.enter_context(tc.tile_pool(name="tailp", bufs=3))

            for p_ in range(NPAIR):
                acc0 = accp.tile([HD + 1, TL], F32, tag="acc", name="acc0")
                acc1 = accp.tile([HD + 1, TL], F32, tag="acc", name="acc1")
                for c in range(R * NTT):
                    r, s = divmod(c, NTT)
                    kt = kTf[p_][r]
                    sc = scp.tile([P, 2 * TL], F32, tag="sc", name="sc")
                    nc.tensor.matmul(sc[:, 0:TL],
                                     kt[0:HD, P * s:P * (s + 1)],
                                     qT[p_][0:HD, :], start=True, stop=True)
                    nc.tensor.matmul(sc[:, TL:2 * TL],
                                     kt[HD:P, P * s:P * (s + 1)],
                                     qT[p_][HD:P, :], start=True, stop=True)
                    wei = weip.tile([P, 2 * TL], BF16, tag="wei", name="wei")
                    if c % 7 in EXP_DVE_RESIDUES:
                        nc.vector._custom_dve(EXP3, out=wei[:], in0=sc[:],
                                              s0=EXP_K1, s1=EXP_K2, imm2=EXP_K3)
                    else:
                        nc.scalar.activation(wei[:], sc[:], AFT.Exp)
                    vt = vf[p_][r]
                    nc.tensor.matmul(acc0[:],
                                     vt[:, PW * s:PW * s + HD + 1],
                                     wei[:, 0:TL],
                                     start=(c == 0), stop=(c == R * NTT - 1))
                    nc.tensor.matmul(acc1[:],
                                     vt[:, PW * s + HD + 1:PW * (s + 1)],
                                     wei[:, TL:2 * TL],
                                     start=(c == 0), stop=(c == R * NTT - 1))

                # softmax tail: den -> 1/den -> broadcast -> scale
                from concourse.dve_ops import (
                    RECIPROCAL_APPROX_FAST, RECIPROCAL_APPROX_NR,
                    RECIP_APPROX_FAST_CONSTS as _RC)
                for hh, acc in ((0, acc0), (1, acc1)):
                    denc = tailp.tile([1, TL], F32, tag="denc", name="denc")
                    nc.vector.tensor_copy(denc[:], acc[HD:HD + 1, :])
                    den = tailp.tile([1, TL], F32R, tag="den", name="den")
                    scr = tailp.tile([1, TL], F32, tag="scr", name="scr")
                    nc.vector._custom_dve(
                        RECIPROCAL_APPROX_FAST, out=scr[:],
                        in0=denc[:], s0=_RC["s0"], s1=_RC["s1"],
                        imm2=_RC["imm2"])
                    nc.vector._custom_dve(
                        RECIPROCAL_APPROX_NR, out=den[:],
                        in0=denc[:], in1=scr[:], s0=2.0)
                    bc = scp.tile([HD, TL], F32, tag="sc", name="bc")
                    nc.tensor.matmul(bc[:], ones_row[:, 0:HD], den[:],
                                     start=True, stop=True)
                    recb = tailp.tile([HD, TL], F32, tag="recb", name="recb")
                    nc.vector.tensor_copy(recb[:], bc[:])
                    nc.vector.tensor_tensor(
                        attnT[p_][HD * hh:HD * (hh + 1), :], acc[0:HD, :],
                        recb[:], mybir.AluOpType.mult)

        kv_scope.close()

        # ------------------------------------------------------------------
        # Phase 3: FFN1 and FFN2 interleaved.
        #   h^T[f, t] = gelu(W1^T attn^T + b1)      (per f-tile)
        #   out^T[o, t] += W2[f, o]^T h^T[f, t]     (accumulated over f)
        # ------------------------------------------------------------------
        w_ff1_v = w_ff1.ap().rearrange("(o p) f -> p o f", p=P)
        w_ff2_v = w_ff2.ap().rearrange("(o p) d -> p o d", p=P)
        with ExitStack() as ph3:
            ps1 = ph3.enter_context(tc.tile_pool(name="ps1", bufs=2, space="PSUM"))
            ps2 = ph3.enter_context(tc.tile_pool(name="ps2", bufs=1, space="PSUM"))
            outp = ph3.enter_context(tc.tile_pool(name="outp", bufs=2))
            acc2 = [ps2.tile([P, TL], F32, name=f"acc2_{g}") for g in range(NDT)]
            for ft in range(NFT):
                w1 = w1p.tile([P, NDT, P], BF16, tag="w1", name="w1")
                nc.gpsimd.dma_start(w1[:], w_ff1_v[:, :, P * ft:P * (ft + 1)])
                ps = ps1.tile([P, TL], F32, tag="ps1t", name="ps1t")
                for d_ in range(NDT):
                    nc.tensor.matmul(ps[:], w1[:, d_, :], attnT[d_][:],
                                     start=(d_ == 0), stop=(d_ == NDT - 1))
                nc.scalar.activation(hT[ft][:], ps[:], AFT.Gelu,
                                     bias=b1_sb[:, ft:ft + 1])
                w2 = w2p.tile([P, D], BF16, tag="w2", name="w2")
                nc.gpsimd.dma_start(w2[:], w_ff2_v[:, ft, :])
                for ot in range(NDT):
                    nc.tensor.matmul(acc2[ot][:],
                                     w2[:, P * ot:P * (ot + 1)],
                                     hT[ft][:],
                                     start=(ft == 0), stop=(ft == NFT - 1))
            for ot in range(NDT):
                outT = outp.tile([P, TL], F32, tag="outT", name="outT")
                nc.vector.tensor_scalar_add(outT[:], acc2[ot][:],
                                            b2_sb[:, ot:ot + 1])
                nc.scalar.dma_start(y.ap()[P * ot:P * (ot + 1), :], outT[:])

    # The Tile path never runs bacc's codegen_inst_isa_subclasses pass, so
    # custom-DVE ISA wrappers would serialize with empty instruction bytes
    # ("ISA wrong length" in walrus). Lower them in place here.
    import concourse.bass_isa as bass_isa
    for func in nc.m.functions:
        for blk in func.blocks:
            i = 0
            while i < len(blk.instructions):
                inst = blk.instructions[i]
                if isinstance(inst, bass_isa.InstCustomDveAnt):
                    lowered = mybir.codegen_inst_isa_one(inst, nc._state, nc.isa)
                    assert isinstance(lowered, list)
                    del nc.inst_map[inst.name]
                    blk.instructions[i:i + 1] = lowered
                    for li in lowered:
                        nc.inst_map[li.name] = li
                    i += len(lowered)
                else:
                    i += 1

    return nc


def _get_nc():
    if "nc" not in _NC_CACHE:
        _NC_CACHE["nc"] = _build_nc()
    return _NC_CACHE["nc"]


def run_sharded(inputs, **run_kwargs):
    """Run the SPMD kernel; returns (full_output [1,4096,768], BassKernelResults)."""
    x = np.asarray(inputs["x"], dtype=np.float32)
    assert x.shape == (1, T, D), x.shape

    w_qkv = np.asarray(inputs["w_qkv"], dtype=np.float32).copy()
    b_qkv = np.asarray(inputs["b_qkv"], dtype=np.float32).copy()
    # fold the score scale into the q projection (bf16); k stays unscaled so
    # its fp8 values sit in the format's normal range
    w_qkv[:, 0:D] *= SCALE
    b_qkv[0:D] *= SCALE

    common = {
        "w_qkv": np.ascontiguousarray(w_qkv.astype(ml_dtypes.bfloat16)),
        "b_qkv": np.ascontiguousarray(b_qkv),
        "w_ff1": np.ascontiguousarray(
            np.asarray(inputs["w_ff1"], dtype=np.float32).astype(ml_dtypes.bfloat16)),
        "b_ff1": np.ascontiguousarray(np.asarray(inputs["b_ff1"], dtype=np.float32)),
        "w_ff2": np.ascontiguousarray(
            np.asarray(inputs["w_ff2"], dtype=np.float32).astype(ml_dtypes.bfloat16)),
        "b_ff2": np.ascontiguousarray(np.asarray(inputs["b_ff2"], dtype=np.float32)),
    }
    x_bf = x.astype(ml_dtypes.bfloat16)
    in_maps = []
    for r in range(R):
        m = dict(common)
        m["x"] = np.ascontiguousarray(x_bf[0, TL * r:TL * (r + 1), :])
        in_maps.append(m)
    nc = _get_nc()
    res = run_bass_kernel_spmd(nc, in_maps, core_ids=list(range(R)), **run_kwargs)
    out = np.concatenate([res.results[r]["y"].T for r in range(R)], axis=0)
    return out.reshape(1, T, D), res


def kernel(**inputs):
    out, _ = run_sharded(inputs)
    return out
